# revision 17
# baseline (speedup 1.0000x reference)
"""Trainium2 Bass kernel for dynamic-scale FP8 GEMM (MixLinear):

    out = (scale_in * scale_w) * (q8(x / scale_in) @ q8(w).T) + bias
    scale_in = max|x| / 448  (global over the whole activation tensor)

Strategy (8 NeuronCores, SPMD):
  - Data-parallel over M = B*S = 16384: each core gets a 2048-row shard of x,
    full weight + bias (replicated).
  - On-device global amax: per-core abs-max reduce, then an AllGather of the
    8 per-core maxima + local max (cheaper than AllReduce).
  - TRN fp8_e4m3 saturates at +-240 (vs OCP e4m3fn's +-448), so quantize with
    a 2x scale (values land in +-224) and fold the 2x back at dequant time.
  - x is DMA-transposed (fp16, xbar path) into [K-partition, K/128, M] layout
    and quantized on-chip; the weight is host-pretransposed to [K, N] (static
    layout prep) and loaded+quantized by a single casting SWDGE DMA.  The
    GEMM runs in fp8 DoubleRow perf mode (contraction 256 per matmul).
  - Xbar transposes must stay on ONE queue (concurrent transposes corrupt),
    and Tile serializes transpose<->copy transitions globally, so the x
    transposes run as one clean burst ordered before every other DMA.
  - PSUM is evicted with a single ScalarE activation: out = psum*2s + bias
    (output kept N-major: psum partitions = N-tile), so bias is a per-partition
    scalar.  Per-core output is [N, M_shard]; the host transposes on gather.
"""

import os
import sys

try:
    import concourse  # noqa: F401
except ImportError:  # pragma: no cover
    for _p in ("/opt/trn_rl_repo", "/root/.axon_site/_ro/trn_rl_repo"):
        if os.path.isdir(_p) and _p not in sys.path:
            sys.path.insert(0, _p)

import numpy as np

import concourse.bacc as bacc
import concourse.bass as bass  # noqa: F401
import concourse.mybir as mybir
import concourse.tile as tile
from concourse import bass_isa
from concourse.bass_utils import run_bass_kernel_spmd

# Problem shapes (hardcoded per contract).
B, S, K, N = 4, 4096, 2048, 2048
M = B * S
N_CORES = 8
MS = M // N_CORES  # 2048 rows of x per core

P = 128
F16 = mybir.dt.float16
F32 = mybir.dt.float32
FP8 = mybir.dt.float8e4


def build_nc(ms=MS, k=K, n=N, n_cores=N_CORES):
    """Build + compile the per-core Bass program (SPMD: same NEFF on all cores)."""
    ko = k // P          # k-outer planes
    assert k % 256 == 0 and ms % 1024 == 0 and n % 256 == 0
    m_chunks = ms // 512     # x transpose/quant chunk count (512 m each)
    nt_tiles = n // P        # GEMM stationary n-tiles
    k_pairs = ko // 2        # DoubleRow k steps

    nc = bacc.Bacc("TRN2", target_bir_lowering=False, debug=False, num_devices=n_cores)
    x = nc.dram_tensor("x", [ms, k], F16, kind="ExternalInput")
    wt = nc.dram_tensor("wt", [k, n], F16, kind="ExternalInput")
    b = nc.dram_tensor("b", [n], F16, kind="ExternalInput")
    out_t = nc.dram_tensor("out_t", [n, ms], F16, kind="ExternalOutput")

    with tile.TileContext(nc) as tc:
        with (
            tc.tile_pool(name="big", bufs=1) as big,
            tc.tile_pool(name="small", bufs=1) as small,
            tc.tile_pool(name="ev", bufs=6) as ev,
            tc.tile_pool(name="psum", bufs=2, space="PSUM") as psum,
            tc.tile_pool(name="dram", bufs=1, space="DRAM") as dram,
        ):
            # Persistent SBUF tensors.
            xT = big.tile([P, m_chunks, ko, 512], F16)  # x^T, chunk-major
            xq = big.tile([P, ko, ms], FP8)    # quantized x (scale 2s)
            wq = big.tile([P, ko, n], FP8)     # quantized w (scale 1)

            # ---- Phase A ------------------------------------------------
            # Xbar transposes of x: one clean burst on the Sync queue.
            # (Transposes corrupt if issued from two queues concurrently,
            # and Tile serializes transpose<->copy transitions globally, so
            # every DRAM->SBUF copy and the collective are ordered after the
            # burst.)  DVE abs-max reduces trail each chunk.
            acc_cols = small.tile([P, m_chunks * 2], F32)
            tr_insts = []
            for mc in range(m_chunks):
                ti = nc.sync.dma_start(
                    out=xT[:, mc],
                    in_=x.ap()[mc * 512:(mc + 1) * 512, :],
                    transpose=True,
                )
                tr_insts.append(ti)
                for hh in range(2):
                    nc.vector.tensor_reduce(
                        acc_cols[:, mc * 2 + hh:mc * 2 + hh + 1],
                        xT[:, mc, :, hh * 256:(hh + 1) * 256],
                        axis=mybir.AxisListType.XY,
                        op=mybir.AluOpType.max,
                        apply_absolute_value=True,
                    )
            amax_col = small.tile([P, 1], F32)
            nc.vector.tensor_reduce(
                amax_col, acc_cols[:], axis=mybir.AxisListType.X,
                op=mybir.AluOpType.max,
            )
            amax_all = small.tile([P, 1], F32)
            nc.gpsimd.partition_all_reduce(
                amax_all, amax_col, channels=P, reduce_op=bass_isa.ReduceOp.max
            )

            # ---- AllGather amaxes across cores, reduce locally ------------
            cc_in = dram.tile([1], F32)
            cc_addr = "Shared" if n_cores > 4 else "Local"
            cc_out = dram.tile([n_cores], F32, addr_space=cc_addr)
            cci = nc.scalar.dma_start(cc_in[:], amax_all[0:1, 0])
            tile.add_dep_helper(
                cci.ins, tr_insts[-1].ins,
                reason="xbar: cc staging after transpose burst",
            )
            nc.gpsimd.collective_compute(
                "AllGather",
                mybir.AluOpType.bypass,
                replica_groups=[list(range(n_cores))],
                ins=[cc_in.opt()],
                outs=[cc_out.opt()],
            )
            # ---- Phase W: weight load+quantize via SWDGE cast-DMA ---------
            # wt DRAM is already [K, N]; SWDGE casts fp16->fp8e4 (RNE,
            # verified) during the transfer.  k = j*128 + p matches the
            # xbar-transpose k-mapping used for x.  Chunked so the GEMM can
            # start on the first n-range; ordered after the transpose burst
            # (DRAM->SBUF copies conflict with xbar-transpose mode).
            for i in range(4):
                n0 = i * (n // 4)
                wi = nc.gpsimd.dma_start(
                    out=wq[:, :, n0:n0 + n // 4],
                    in_=wt.ap()[:, n0:n0 + n // 4].rearrange(
                        "(j p) n2 -> p j n2", p=P
                    ),
                )
                tile.add_dep_helper(
                    wi.ins, tr_insts[-1].ins,
                    reason="xbar: weight copy after transpose burst",
                )

            # bias -> SBUF [128, n/128] fp32, [p, j] = bias[j*128 + p]
            bias16 = small.tile([P, nt_tiles], F16)
            bi = nc.scalar.dma_start(
                bias16[:], b.ap().rearrange("(j p) -> p j", p=P)
            )
            tile.add_dep_helper(
                bi.ins, tr_insts[-1].ins,
                reason="xbar: bias copy after transpose burst",
            )
            bias32 = small.tile([P, nt_tiles], F32)
            nc.vector.tensor_copy(bias32[:], bias16[:])

            scal0 = small.tile([P, n_cores], F32)
            nc.scalar.dma_start(scal0[0:1, :], cc_out[:])
            amax1 = small.tile([P, 1], F32)
            nc.vector.tensor_reduce(
                amax1[0:1, :], scal0[0:1, :], axis=mybir.AxisListType.X,
                op=mybir.AluOpType.max,
            )
            amax_bc = small.tile([P, 1], F32)
            nc.gpsimd.partition_broadcast(amax_bc, amax1[0:1, :], channels=P)

            # inv2s = 224/amax (quant scale), s2 = amax/224 (dequant scale)
            inv_amax = small.tile([P, 1], F32)
            nc.vector.reciprocal(inv_amax, amax_bc)
            inv2s = small.tile([P, 1], F32)
            nc.vector.tensor_scalar_mul(inv2s, inv_amax, 224.0)
            s2 = small.tile([P, 1], F32)
            nc.vector.tensor_scalar_mul(s2, amax_bc, 1.0 / 224.0)

            # ---- Phases Q+G interleaved: quantize a 512-m quarter, GEMM it.
            # Quantization alternates VectorE (tensor_scalar, ~2x mode) and
            # ScalarE (activation w/ scale) per quarter.
            for mq in range(ms // 512):
                sl = slice(mq * 512, (mq + 1) * 512)
                if mq % 2 == 0:
                    nc.vector.tensor_scalar(
                        xq[:, :, sl], xT[:, mq], inv2s[:], None,
                        mybir.AluOpType.mult,
                    )
                else:
                    nc.scalar.activation(
                        xq[:, :, sl], xT[:, mq],
                        mybir.ActivationFunctionType.Copy, scale=inv2s[:],
                    )
                m0 = mq * 512
                for nt in range(nt_tiles):
                    ps = psum.tile(
                        [P, 512], F32, tag="ps", bufs=6, name=f"ps_{mq}_{nt}"
                    )
                    for k8 in range(k_pairs):
                        nc.tensor.matmul(
                            ps[:],
                            lhsT=wq[:, 2 * k8:2 * k8 + 2, nt * P:(nt + 1) * P],
                            rhs=xq[:, 2 * k8:2 * k8 + 2, m0:m0 + 512],
                            start=(k8 == 0),
                            stop=(k8 == k_pairs - 1),
                            perf_mode=mybir.MatmulPerfMode.DoubleRow,
                        )
                    ob = ev.tile([P, 512], F16, tag="ob", name=f"ob_{mq}_{nt}")
                    nc.scalar.activation(
                        ob[:], ps[:],
                        mybir.ActivationFunctionType.Identity,
                        bias=bias32[:, nt:nt + 1],
                        scale=s2[:],
                    )
                    nc.sync.dma_start(
                        out_t.ap()[nt * P:(nt + 1) * P, m0:m0 + 512], ob[:]
                    )

    nc.compile()
    return nc


_NC_CACHE = {}


def _get_nc():
    if "nc" not in _NC_CACHE:
        _NC_CACHE["nc"] = build_nc()
    return _NC_CACHE["nc"]


def kernel(x, weight, bias):
    x = np.asarray(x, dtype=np.float16).reshape(M, K)
    weight = np.asarray(weight, dtype=np.float16)
    bias = np.asarray(bias, dtype=np.float16)

    nc = _get_nc()
    wt = np.ascontiguousarray(weight.T)  # [K, N] — static-weight layout prep
    in_maps = [
        {"x": x[c * MS:(c + 1) * MS], "wt": wt, "b": bias}
        for c in range(N_CORES)
    ]
    trace = bool(int(os.environ.get("KERNEL_TRACE", "0")))
    res = run_bass_kernel_spmd(nc, in_maps, list(range(N_CORES)), trace=trace)
    _NC_CACHE["last_result"] = res

    out = np.empty((M, N), dtype=np.float16)
    for c in range(N_CORES):
        out[c * MS:(c + 1) * MS, :] = res.results[c]["out_t"].T
    return out.reshape(B, S, N)


# revision 18
# speedup vs baseline: 1.0441x; 1.0441x over previous
"""Trainium2 Bass kernel for dynamic-scale FP8 GEMM (MixLinear):

    out = (scale_in * scale_w) * (q8(x / scale_in) @ q8(w).T) + bias
    scale_in = max|x| / 448  (global over the whole activation tensor)

Strategy (8 NeuronCores, SPMD):
  - Data-parallel over M = B*S = 16384: each core gets a 2048-row shard of x,
    full weight + bias (replicated).
  - On-device global amax: per-core abs-max reduce, then an AllGather of the
    8 per-core maxima + local max (cheaper than AllReduce).
  - TRN fp8_e4m3 saturates at +-240 (vs OCP e4m3fn's +-448), so quantize with
    a 2x scale (values land in +-224) and fold the 2x back at dequant time.
  - x is DMA-transposed (fp16, xbar path) into [K-partition, K/128, M] layout
    and quantized on-chip; the weight is host-pretransposed to [K, N] (static
    layout prep) and loaded+quantized by a single casting SWDGE DMA.  The
    GEMM runs in fp8 DoubleRow perf mode (contraction 256 per matmul).
  - Xbar transposes must stay on ONE queue (concurrent transposes corrupt),
    and Tile serializes transpose<->copy transitions globally, so the x
    transposes run as one clean burst ordered before every other DMA.
  - PSUM is evicted with a single ScalarE activation: out = psum*2s + bias
    (output kept N-major: psum partitions = N-tile), so bias is a per-partition
    scalar.  Per-core output is [N, M_shard]; the host transposes on gather.
"""

import os
import sys

try:
    import concourse  # noqa: F401
except ImportError:  # pragma: no cover
    for _p in ("/opt/trn_rl_repo", "/root/.axon_site/_ro/trn_rl_repo"):
        if os.path.isdir(_p) and _p not in sys.path:
            sys.path.insert(0, _p)

import numpy as np

import concourse.bacc as bacc
import concourse.bass as bass  # noqa: F401
import concourse.mybir as mybir
import concourse.tile as tile
from concourse import bass_isa
from concourse.bass_utils import run_bass_kernel_spmd

# Problem shapes (hardcoded per contract).
B, S, K, N = 4, 4096, 2048, 2048
M = B * S
N_CORES = 8
MS = M // N_CORES  # 2048 rows of x per core

P = 128
F16 = mybir.dt.float16
F32 = mybir.dt.float32
FP8 = mybir.dt.float8e4


def build_nc(ms=MS, k=K, n=N, n_cores=N_CORES):
    """Build + compile the per-core Bass program (SPMD: same NEFF on all cores)."""
    ko = k // P          # k-outer planes
    assert k % 256 == 0 and ms % 1024 == 0 and n % 256 == 0
    m_chunks = ms // 512     # x transpose/quant chunk count (512 m each)
    nt_tiles = n // P        # GEMM stationary n-tiles
    k_pairs = ko // 2        # DoubleRow k steps

    nc = bacc.Bacc("TRN2", target_bir_lowering=False, debug=False, num_devices=n_cores)
    x = nc.dram_tensor("x", [ms, k], F16, kind="ExternalInput")
    wt = nc.dram_tensor("wt", [k, n], F16, kind="ExternalInput")
    b = nc.dram_tensor("b", [n], F16, kind="ExternalInput")
    out_t = nc.dram_tensor("out_t", [n, ms], F16, kind="ExternalOutput")

    with tile.TileContext(nc) as tc:
        with (
            tc.tile_pool(name="big", bufs=1) as big,
            tc.tile_pool(name="small", bufs=1) as small,
            tc.tile_pool(name="ev", bufs=6) as ev,
            tc.tile_pool(name="psum", bufs=2, space="PSUM") as psum,
            tc.tile_pool(name="dram", bufs=1, space="DRAM") as dram,
        ):
            # Persistent SBUF tensors.
            xT = big.tile([P, m_chunks, ko, 512], F16)  # x^T, chunk-major
            xq = big.tile([P, ko, ms], FP8)    # quantized x (scale 2s)
            wq = big.tile([P, ko, n], FP8)     # quantized w (scale 1)

            # ---- Phase A ------------------------------------------------
            # Xbar transposes of x: one clean burst on the Sync queue.
            # (Transposes corrupt if issued from two queues concurrently,
            # and Tile serializes transpose<->copy transitions globally, so
            # every DRAM->SBUF copy and the collective are ordered after the
            # burst.)  DVE abs-max reduces trail each chunk.
            acc_cols = small.tile([P, m_chunks * 2], F32)
            tr_insts = []
            for mc in range(m_chunks):
                ti = nc.sync.dma_start(
                    out=xT[:, mc],
                    in_=x.ap()[mc * 512:(mc + 1) * 512, :],
                    transpose=True,
                )
                tr_insts.append(ti)
                for hh in range(2):
                    nc.vector.tensor_reduce(
                        acc_cols[:, mc * 2 + hh:mc * 2 + hh + 1],
                        xT[:, mc, :, hh * 256:(hh + 1) * 256],
                        axis=mybir.AxisListType.XY,
                        op=mybir.AluOpType.max,
                        apply_absolute_value=True,
                    )
            amax_col = small.tile([P, 1], F32)
            nc.vector.tensor_reduce(
                amax_col, acc_cols[:], axis=mybir.AxisListType.X,
                op=mybir.AluOpType.max,
            )
            amax_all = small.tile([P, 1], F32)
            nc.gpsimd.partition_all_reduce(
                amax_all, amax_col, channels=P, reduce_op=bass_isa.ReduceOp.max
            )

            # ---- AllGather amaxes across cores, reduce locally ------------
            cc_in = dram.tile([1], F32)
            cc_addr = "Shared" if n_cores > 4 else "Local"
            cc_out = dram.tile([n_cores], F32, addr_space=cc_addr)
            cci = nc.scalar.dma_start(cc_in[:], amax_all[0:1, 0])
            tile.add_dep_helper(
                cci.ins, tr_insts[-1].ins,
                reason="xbar: cc staging after transpose burst",
            )
            nc.gpsimd.collective_compute(
                "AllGather",
                mybir.AluOpType.bypass,
                replica_groups=[list(range(n_cores))],
                ins=[cc_in.opt()],
                outs=[cc_out.opt()],
            )
            # ---- Phase W: weight load+quantize via SWDGE cast-DMA ---------
            # wt DRAM is already [K, N]; SWDGE casts fp16->fp8e4 (RNE,
            # verified) during the transfer.  k = j*128 + p matches the
            # xbar-transpose k-mapping used for x.  Chunked so the GEMM can
            # start on the first n-range; ordered after the transpose burst
            # (DRAM->SBUF copies conflict with xbar-transpose mode).
            for i in range(4):
                n0 = i * (n // 4)
                wi = nc.gpsimd.dma_start(
                    out=wq[:, :, n0:n0 + n // 4],
                    in_=wt.ap()[:, n0:n0 + n // 4].rearrange(
                        "(j p) n2 -> p j n2", p=P
                    ),
                )
                tile.add_dep_helper(
                    wi.ins, tr_insts[-1].ins,
                    reason="xbar: weight copy after transpose burst",
                )

            # bias -> SBUF [128, n/128] fp32, [p, j] = bias[j*128 + p]
            bias16 = small.tile([P, nt_tiles], F16)
            bi = nc.scalar.dma_start(
                bias16[:], b.ap().rearrange("(j p) -> p j", p=P)
            )
            tile.add_dep_helper(
                bi.ins, tr_insts[-1].ins,
                reason="xbar: bias copy after transpose burst",
            )
            bias32 = small.tile([P, nt_tiles], F32)
            nc.vector.tensor_copy(bias32[:], bias16[:])

            scal0 = small.tile([P, n_cores], F32)
            nc.scalar.dma_start(scal0[0:1, :], cc_out[:])
            amax1 = small.tile([P, 1], F32)
            nc.vector.tensor_reduce(
                amax1[0:1, :], scal0[0:1, :], axis=mybir.AxisListType.X,
                op=mybir.AluOpType.max,
            )
            amax_bc = small.tile([P, 1], F32)
            nc.gpsimd.partition_broadcast(amax_bc, amax1[0:1, :], channels=P)

            # inv2s = 224/amax (quant scale), s2 = amax/224 (dequant scale)
            inv_amax = small.tile([P, 1], F32)
            nc.vector.reciprocal(inv_amax, amax_bc)
            inv2s = small.tile([P, 1], F32)
            nc.vector.tensor_scalar_mul(inv2s, inv_amax, 224.0)
            s2 = small.tile([P, 1], F32)
            nc.vector.tensor_scalar_mul(s2, amax_bc, 1.0 / 224.0)

            # ---- Phases Q+G interleaved: quantize a 512-m quarter, GEMM it.
            # Quantization alternates VectorE (tensor_scalar, ~2x mode) and
            # ScalarE (activation w/ scale) per quarter.
            for mq in range(ms // 512):
                h0 = slice(mq * 512, mq * 512 + 256)
                h1 = slice(mq * 512 + 256, (mq + 1) * 512)
                nc.vector.tensor_scalar(
                    xq[:, :, h0], xT[:, mq, :, 0:256], inv2s[:], None,
                    mybir.AluOpType.mult,
                )
                nc.scalar.activation(
                    xq[:, :, h1], xT[:, mq, :, 256:512],
                    mybir.ActivationFunctionType.Copy, scale=inv2s[:],
                )
                m0 = mq * 512
                for nt in range(nt_tiles):
                    ps = psum.tile(
                        [P, 512], F32, tag="ps", bufs=7, name=f"ps_{mq}_{nt}"
                    )
                    for k8 in range(k_pairs):
                        nc.tensor.matmul(
                            ps[:],
                            lhsT=wq[:, 2 * k8:2 * k8 + 2, nt * P:(nt + 1) * P],
                            rhs=xq[:, 2 * k8:2 * k8 + 2, m0:m0 + 512],
                            start=(k8 == 0),
                            stop=(k8 == k_pairs - 1),
                            perf_mode=mybir.MatmulPerfMode.DoubleRow,
                        )
                    ob = ev.tile([P, 512], F16, tag="ob", name=f"ob_{mq}_{nt}")
                    nc.scalar.activation(
                        ob[:], ps[:],
                        mybir.ActivationFunctionType.Identity,
                        bias=bias32[:, nt:nt + 1],
                        scale=s2[:],
                    )
                    nc.sync.dma_start(
                        out_t.ap()[nt * P:(nt + 1) * P, m0:m0 + 512], ob[:]
                    )

    nc.compile()
    return nc


_NC_CACHE = {}


def _get_nc():
    if "nc" not in _NC_CACHE:
        _NC_CACHE["nc"] = build_nc()
    return _NC_CACHE["nc"]


def kernel(x, weight, bias):
    x = np.asarray(x, dtype=np.float16).reshape(M, K)
    weight = np.asarray(weight, dtype=np.float16)
    bias = np.asarray(bias, dtype=np.float16)

    nc = _get_nc()
    wt = np.ascontiguousarray(weight.T)  # [K, N] — static-weight layout prep
    in_maps = [
        {"x": x[c * MS:(c + 1) * MS], "wt": wt, "b": bias}
        for c in range(N_CORES)
    ]
    trace = bool(int(os.environ.get("KERNEL_TRACE", "0")))
    res = run_bass_kernel_spmd(nc, in_maps, list(range(N_CORES)), trace=trace)
    _NC_CACHE["last_result"] = res

    out = np.empty((M, N), dtype=np.float16)
    for c in range(N_CORES):
        out[c * MS:(c + 1) * MS, :] = res.results[c]["out_t"].T
    return out.reshape(B, S, N)
